# revision 1
# baseline (speedup 1.0000x reference)
"""Trainium2 Bass kernel for the BSDE solver (nn_BSDESolver).

Math (per path, M=50 steps, a = 1+R*DT):
  S_{i+1} = S_i * g_i,  g_i = 1 + R*DT + SIGMA*dw_i     (z-independent GBM)
  Y_M = a^M Y0 + sum_i a^(M-1-i) * SIGMA * S_i * dw_i * z_i    (linear in z)

z_i = MLP(S_i/S0, t_i) where t_i is a per-step constant, so z_i is a smooth
scalar function of x = S_i/S0 per step.  Host-side we fit a per-step basis
z_s(x) ~ b/x + b0 + b2 x^2 (near-minimax fit err ~7e-3 over +-7.5 sigma of
log S), chosen so that w*z needs only dw-multiplied tiles
  dw (free!),  w = x*dw,  P1 = w*x,  P2 = P1*x        (3 DVE multiplies)
and just THREE accumulated contraction matmuls per 512-path block (rhs
tiles dw, w, P2) with coefficients gamma_{k,s} = SIGMA*S0*a^(49-s)*beta_{s,k}.

Per 2048 paths (a "quad": 4 column-blocks of 512, two 50-step row-groups
packed into 100 partitions):
  PE  : 4 prefix matmuls (triangular const, bf16) + 12 contraction matmuls
        (bf16, accumulated into 4-row regions of a [64,512] PSUM tile)
  ACT : quarter-granular Ln(sigma*dw + 1+R*DT) of the NEXT slab interleaved
        between 2x Exp(prefix) (+ln(S0) bias on the two total-rows so S_50
        rides along in rows 100-101 of the x tile)
  DVE : the 3-multiply chain at 2048 cols, all bf16 (2x mode), plus the
        [64,512] PSUM->SBUF staging copy per 16 blocks
GpSimd is deliberately unused: its tensor ops are Q7 software loops measured
~20x slower than the cost model claims on this hardware.

Data parallel over batch across 8 cores; step-major bf16 layout built
host-side; dw ships as bf16 (halves HBM traffic).
"""
import numpy as np

import concourse.mybir as mybir
import concourse.tile as tile
import concourse.bacc as bacc
from concourse import bass_utils

F32 = mybir.dt.float32
BF16 = mybir.dt.bfloat16
AF = mybir.ActivationFunctionType
ALU = mybir.AluOpType

S0, R, SIGMA, T = 100.0, 0.05, 0.2, 1.0
M = 50
DT = T / M
RDT = R * DT
A = 1.0 + RDT
LNS0 = float(np.log(S0))
NCORES = 8
B_FULL = 1048576
B_CORE = B_FULL // NCORES          # 131072 paths
NDB = B_CORE // 1024               # 128 double-blocks of 1024 paths
NQD = NDB // 4                     # 32 quads of 4 double-blocks
NSLAB = NQD // 2                   # 16 slabs of 2 quads
LQ = 3                             # contraction lookahead (quads)


def _fit_beta(W1, b1, W2, b2, W3, b3, ts):
    """Per-step fit of z_s(x) in basis {x^-1, 1, x^2}, x = S/S0, on a
    Chebyshev grid of u = log x covering +-7.5 sigma of the step's
    log-price distribution."""
    sdt = SIGMA * np.sqrt(DT)
    beta = np.zeros((M, 3), np.float64)
    th = np.linspace(0.0, np.pi, 801)
    grid01 = 0.5 * (1.0 - np.cos(th))
    for s in range(M):
        std = sdt * np.sqrt(max(s, 1))
        drift = s * (RDT - 0.5 * SIGMA * SIGMA * DT)
        half = max(7.5 * std, 0.02)
        u = (drift - half) + 2.0 * half * grid01
        x = np.exp(u)
        h = np.tanh(np.stack([x, np.full_like(x, ts[s])], 1) @ W1 + b1)
        h = np.tanh(h @ W2 + b2)
        z = 1.0 / (1.0 + np.exp(-(h @ W3 + b3)))[:, 0]
        # basis {x^-1, 1, x^2}: x^-1 contracts the raw dw tile and x^2 the
        # P2 = dw*x^3 tile, so only 3 contraction matmuls + 3 multiplies.
        # IRLS sharpens the LS fit toward minimax.
        Am = np.stack([1.0 / x, np.ones_like(x), x * x], 1)
        wgt = np.ones_like(z)
        for _ in range(6):
            c, *_ = np.linalg.lstsq(Am * wgt[:, None], z * wgt, rcond=None)
            r = np.abs(Am @ c - z)
            wgt = np.sqrt(wgt * np.maximum(r / max(r.max(), 1e-12), 1e-3))
        beta[s] = c
    return beta


def _build_consts(W1, b1, W2, b2, W3, b3, ts):
    import ml_dtypes
    beta = _fit_beta(W1, b1, W2, b2, W3, b3, ts)
    gam = SIGMA * S0 * A ** (49 - np.arange(M, dtype=np.float64))

    TRIZ = np.zeros((100, 102), np.float32)
    for c in range(50):
        TRIZ[:c, c] = 1.0                 # u_A[c] = sum_{r<c} lg_A[r]
        TRIZ[50:50 + c, 50 + c] = 1.0     # u_B
    TRIZ[0:50, 100] = 1.0                 # total A
    TRIZ[50:100, 101] = 1.0               # total B

    # CB[k] contracts rhs tile k: 0 -> dw (x^-1 term), 1 -> w (S_50 pick
    # rides here via w rows 100-101 = S_50), 2 -> P2 (x^2 term)
    CB = np.zeros((3, 102, 1024), np.float32)
    for a in range(16):
        for k in range(3):
            CB[k, 0:50, 64 * a + 4 * a] = gam * beta[:, k]
            CB[k, 50:100, 64 * a + 4 * a + 1] = gam * beta[:, k]
        CB[1, 100, 64 * a + 4 * a + 2] = 1.0
        CB[1, 101, 64 * a + 4 * a + 3] = 1.0

    EB = np.zeros((102, 1), np.float32)
    EB[100:102, 0] = LNS0
    LB = np.full((100, 1), 1.0 + RDT, np.float32)

    c = {"TRIZ": TRIZ.astype(ml_dtypes.bfloat16), "EB": EB, "LB": LB}
    c["CB0"] = CB[0].astype(ml_dtypes.bfloat16)
    c["CB1"] = CB[1].astype(ml_dtypes.bfloat16)
    c["CB2"] = CB[2, 0:100].astype(ml_dtypes.bfloat16)
    return c


def _build_kernel(num_devices, nreps=1):
    nc = bacc.Bacc("TRN2", debug=False, num_devices=num_devices,
                   target_bir_lowering=False)

    # All our activation funcs (ln, exp, copy) live together in the
    # "natural_log_exp_and_others" table, but the default first-match table
    # picker sends exp to "exp_and_others" and ln to "natural_log", inserting
    # ~30 dynamic table reloads (1.3 us each) on the ACT critical path.
    # Claim zero functions for every other table (list positions, and hence
    # act_func_set ids, are unchanged) so one table serves the whole program.
    from concourse.hw_specs import get_activation_tables
    import concourse.bacc as _bacc_mod

    def _pinned_act_table_loads():
        tables = []
        for name, funcs in get_activation_tables(nc.m.arch).items():
            tables.append((name, funcs if name == "natural_log_exp_and_others"
                           else set()))
        _bacc_mod._bass_rust.insert_act_table_loads(nc, tables)

    nc.insert_act_table_loads = _pinned_act_table_loads
    tc = tile.TileContext(nc)

    dwT = nc.dram_tensor("dwT", [102, B_CORE // 2], BF16, kind="ExternalInput")
    cdefs = [("TRIZ", [100, 102], BF16), ("CB0", [102, 1024], BF16),
             ("CB1", [102, 1024], BF16), ("CB2", [100, 1024], BF16),
             ("EB", [102, 1], F32), ("LB", [100, 1], F32)]
    cins = {n: nc.dram_tensor(n, s, d, kind="ExternalInput") for n, s, d in cdefs}
    Zout = nc.dram_tensor("Zout", [512, 512], F32, kind="ExternalOutput")

    with tc:
        with tc.tile_pool(name="consts", bufs=1) as cpool, \
             tc.tile_pool(name="inp", bufs=5) as ipool, \
             tc.tile_pool(name="lgp", bufs=3) as lpool, \
             tc.tile_pool(name="xwp", bufs=LQ + 2) as xpool, \
             tc.tile_pool(name="stg", bufs=2) as spool, \
             tc.tile_pool(name="ps_pref", bufs=3, space="PSUM") as p_pref, \
             tc.tile_pool(name="ps_y", bufs=2, space="PSUM") as p_y:

            C = {}
            for n, s, d in cdefs:
                C[n] = cpool.tile(s, d, name=f"c_{n}", tag=f"c_{n}")
                nc.sync.dma_start(C[n][:], cins[n].ap())

            dwt = {}
            lg = {}
            xq = {}
            wq = {}
            p1q = {}
            p2q = {}
            ypt = {}
            stg = {}

            def dma_in(s):
                dwt[s] = ipool.tile([102, 4096], BF16, name="dwt", tag="dwt")
                nc.sync.dma_start(dwt[s][:],
                                  dwT.ap()[:, s * 4096:(s + 1) * 4096])

            def ln_quarter(s, q):
                # quarter-granular Ln keeps the ACT queue free of 3.6 us
                # head-of-line blockers between the Exp ops
                if q == 0:
                    lg[s] = lpool.tile([100, 4096], BF16, name="lg", tag="lg")
                cols = slice(1024 * q, 1024 * (q + 1))
                nc.scalar.activation(lg[s][:, cols], dwt[s][0:100, cols],
                                     AF.Ln, bias=C["LB"][:], scale=SIGMA)

            def front(t):
                s, qq = t // 2, t % 2
                x = xpool.tile([102, 2048], BF16, name="x", tag="x")
                for h in range(2):
                    pref = p_pref.tile([102, 1024], F32, name="pref", tag="pref")
                    for k2 in range(2):
                        cols = slice(2048 * qq + 1024 * h + 512 * k2,
                                     2048 * qq + 1024 * h + 512 * (k2 + 1))
                        nc.tensor.matmul(pref[:, 512 * k2:512 * (k2 + 1)],
                                         C["TRIZ"][:], lg[s][:, cols],
                                         start=True, stop=True)
                    # a Ln quarter of the next slab in front of each Exp fills
                    # the ACT queue while the Exp waits on its prefix matmuls
                    if s + 1 < NSLAB:
                        ln_quarter(s + 1, 2 * qq + h)
                    nc.scalar.activation(x[:, 1024 * h:1024 * (h + 1)], pref[:],
                                         AF.Exp, bias=C["EB"][:], scale=1.0)
                w = xpool.tile([102, 2048], BF16, name="w", tag="w")
                qcols = slice(2048 * qq, 2048 * (qq + 1))
                nc.vector.tensor_tensor(w[:], x[:], dwt[s][:, qcols],
                                        op=ALU.mult)
                p1 = xpool.tile([100, 2048], BF16, name="p1", tag="p1")
                nc.vector.tensor_tensor(p1[:], w[0:100, :], x[0:100, :],
                                        op=ALU.mult)
                # NOTE: do NOT put any of these on nc.gpsimd — Q7 tensor ops
                # measured ~20x slower than the cost model on this hardware.
                p2 = xpool.tile([100, 2048], BF16, name="p2", tag="p2")
                nc.vector.tensor_tensor(p2[:], p1[:], x[0:100, :], op=ALU.mult)
                xq[t], wq[t], p1q[t], p2q[t] = x, w, p1, p2

            def contract(t):
                hx = t // 4
                if t % 4 == 0:
                    ypt[hx] = p_y.tile([64, 512], F32, name="yp", tag="yp")
                yp = ypt[hx]
                s, qq = t // 2, t % 2
                for j in range(4):
                    a = (4 * t + j) % 16
                    cs = slice(64 * a, 64 * (a + 1))
                    cj = slice(512 * j, 512 * (j + 1))
                    cdw = slice(2048 * qq + 512 * j, 2048 * qq + 512 * (j + 1))
                    first = (t % 4 == 0 and j == 0)
                    last = (t % 4 == 3 and j == 3)
                    nc.tensor.matmul(yp[:], C["CB0"][:, cs], dwt[s][:, cdw],
                                     start=first, stop=False)
                    nc.tensor.matmul(yp[:], C["CB1"][:, cs], wq[t][:, cj],
                                     start=False, stop=False)
                    nc.tensor.matmul(yp[:], C["CB2"][:, cs], p2q[t][:, cj],
                                     start=False, stop=last)
                del xq[t], wq[t], p1q[t], p2q[t]
                if t % 4 == 3:
                    g, h2 = hx // 2, hx % 2
                    if h2 == 0:
                        stg[g] = spool.tile([128, 512], F32, name="st", tag="st")
                    nc.vector.tensor_copy(stg[g][64 * h2:64 * (h2 + 1), :],
                                          yp[:])
                    if h2 == 1:
                        nc.sync.dma_start(
                            Zout.ap()[128 * g:128 * (g + 1), :], stg[g][:])

            for rep in range(nreps):
                dma_in(0)
                dma_in(1)
                for q in range(4):
                    ln_quarter(0, q)
                for t in range(NQD + LQ):
                    if t < NQD:
                        s, qq = t // 2, t % 2
                        if qq == 0 and s + 2 < NSLAB:
                            dma_in(s + 2)
                        front(t)
                    if t >= LQ:
                        contract(t - LQ)

    nc.compile()
    return nc


_CACHE = {}
_LAST_IN_MAPS = None


def kernel(dw, t_grid, W1, b1, W2, b2, W3, b3, Y0):
    import ml_dtypes
    dw = np.asarray(dw, np.float32)
    t_grid = np.asarray(t_grid, np.float32)
    B = dw.shape[0]
    assert B == B_FULL and dw.shape[1] == M
    a50y0 = np.float32(A ** M * np.float32(Y0))

    if "nc" not in _CACHE:
        _CACHE["nc"] = _build_kernel(NCORES)
    nc = _CACHE["nc"]

    consts = _build_consts(np.asarray(W1, np.float32), np.asarray(b1, np.float32),
                           np.asarray(W2, np.float32), np.asarray(b2, np.float32),
                           np.asarray(W3, np.float32), np.asarray(b3, np.float32),
                           t_grid[0])

    dwb = dw.astype(ml_dtypes.bfloat16)
    ones2 = np.ones((2, B_CORE // 2), ml_dtypes.bfloat16)
    in_maps = []
    for ci in range(NCORES):
        blk = dwb[ci * B_CORE:(ci + 1) * B_CORE]
        # [102, B_CORE/2]: col 512*d+c rows 0-49 = steps of path 1024d+c,
        # rows 50-99 = steps of path 1024d+512+c, rows 100-101 = 1.0
        # (so w = x*dw carries S_50 in rows 100-101)
        dwT = np.concatenate([
            blk.reshape(NDB, 2, 512, M).transpose(1, 3, 0, 2).reshape(100, -1),
            ones2], axis=0)
        mci = dict(consts)
        mci["dwT"] = dwT
        in_maps.append(mci)

    global _LAST_IN_MAPS
    _LAST_IN_MAPS = in_maps
    res = bass_utils.run_bass_kernel_spmd(nc, in_maps, core_ids=list(range(NCORES)))

    Y = np.empty((B_FULL,), np.float32)
    S = np.empty((B_FULL,), np.float32)
    for ci in range(NCORES):
        # Zout row = 128*g + 64*h2 + 4*j16 + q, db d = 32g + 16h2 + j16,
        # q in {Y_A, Y_B, S_A, S_B}; path = 1024d + 512*(q%2) + col
        Z = res.results[ci]["Zout"].reshape(4, 2, 16, 4, 512)
        Y[ci * B_CORE:(ci + 1) * B_CORE] = Z[:, :, :, 0:2, :].reshape(-1)
        S[ci * B_CORE:(ci + 1) * B_CORE] = Z[:, :, :, 2:4, :].reshape(-1)
    Y += a50y0
    return Y[:, None], S[:, None]



# revision 6
# speedup vs baseline: 1.5654x; 1.5654x over previous
"""Trainium2 Bass kernel for the BSDE solver (nn_BSDESolver).

Math (per path, M=50 steps, a = 1+R*DT):
  S_{i+1} = S_i * g_i,  g_i = a + SIGMA*dw_i          (z-independent GBM)
  Y_M = a^M Y0 + sum_i gam_i * z_i * x_i * dw_i,  gam_i = SIGMA*S0*a^(49-i)
  x_i = S_i/S0 = exp(u_i),  u_i = sum_{j<i} ln(a + SIGMA*dw_j)

z_i = MLP(x_i, t_i) is fitted per step as z ~ b0/x + b1 + b2 x^2 so that
z*x*dw = b0*dw + b1*w + b2*P2 with w = x*dw, P2 = x^3*dw -- three
contraction matmuls against per-step coefficient columns.

Layout (the big win vs the 100/128-row predecessor): each SBUF column
packs FIVE 25-step path-halves -> 125 of 128 partitions carry data, and
every engine on TRN2 charges by free-dim columns only.  A path's steps
0-24 live in slab "A", steps 25-49 in slab "B" at the same column; B's
prefix adds the A-half total via a second accumulated matmul (TRIZBA =
group-summed ones).  Step 0 has x=1 exactly, so its term is folded as a
constant into CB-A's dw coefficient and TRIZ column 25g+0 is zero.
S_50 = S0*(a*x_49 + SIGMA*w_49) is picked up by one extra matmul on the
x tile (XPB) plus an S0*SIGMA coefficient in CB-B1.

Contraction matmuls have only 10 useful output rows, so the four
512-column blocks of a quad run CONCURRENTLY in the four 32-column
strips of the PE array via tile_position=(0,32jj) -- ~4x on the
contraction part of the PE timeline.

GpSimd is deliberately unused: its tensor ops are Q7 software loops
measured ~20x slower than the cost model claims on this hardware.
Data parallel over batch across 8 cores; dw ships as bf16.
"""
import numpy as np

import concourse.mybir as mybir
import concourse.tile as tile
import concourse.bacc as bacc
from concourse import bass_utils

F32 = mybir.dt.float32
BF16 = mybir.dt.bfloat16
AF = mybir.ActivationFunctionType
ALU = mybir.AluOpType

S0, R, SIGMA, T = 100.0, 0.05, 0.2, 1.0
M = 50
DT = T / M
RDT = R * DT
A = 1.0 + RDT
NCORES = 8
B_FULL = 1048576
B_CORE = B_FULL // NCORES          # 131072 paths
G = 5                              # path-halves packed per column
SH = 25                            # steps per half
NROW = G * SH                      # 125 used partitions
NCOL = 26624                       # ceil(B_CORE/G) padded to 52*512
NQ = NCOL // 2048                  # 13 column quads (each has an A+B slab)
LQ = 3                             # contraction lookahead (quad-slabs)


def _zeta_np(x, t, W1, b1, W2, b2, W3, b3):
    h = np.tanh(np.stack([x, np.full_like(x, t)], 1) @ W1 + b1)
    h = np.tanh(h @ W2 + b2)
    return 1.0 / (1.0 + np.exp(-(h @ W3 + b3)))[:, 0]


def _fit_beta(W1, b1, W2, b2, W3, b3, ts):
    """Per-step fit of z_s(x) in basis {x^-1, 1, x^2}, x = S/S0, on a
    Chebyshev grid of u = log x covering +-7.5 sigma of the step's
    log-price distribution (IRLS sharpens LS toward minimax)."""
    sdt = SIGMA * np.sqrt(DT)
    beta = np.zeros((M, 3), np.float64)
    th = np.linspace(0.0, np.pi, 801)
    grid01 = 0.5 * (1.0 - np.cos(th))
    for s in range(M):
        std = sdt * np.sqrt(max(s, 1))
        drift = s * (RDT - 0.5 * SIGMA * SIGMA * DT)
        half = max(7.5 * std, 0.02)
        u = (drift - half) + 2.0 * half * grid01
        x = np.exp(u)
        z = _zeta_np(x, ts[s], W1, b1, W2, b2, W3, b3)
        Am = np.stack([1.0 / x, np.ones_like(x), x * x], 1)
        wgt = np.ones_like(z)
        for _ in range(6):
            c, *_ = np.linalg.lstsq(Am * wgt[:, None], z * wgt, rcond=None)
            r = np.abs(Am @ c - z)
            wgt = np.sqrt(wgt * np.maximum(r / max(r.max(), 1e-12), 1e-3))
        beta[s] = c
    return beta


def _build_consts(W1, b1, W2, b2, W3, b3, ts):
    import ml_dtypes
    beta = _fit_beta(W1, b1, W2, b2, W3, b3, ts)
    gam = SIGMA * S0 * A ** (49 - np.arange(M, dtype=np.float64))

    # prefix matrices: column 25g+i sums rows 25g+j (j<i) of its own slab;
    # TRIZBA adds the whole A-half of the group into every B prefix.
    TRIZA = np.zeros((NROW, 128), np.float32)
    TRIZB = np.zeros((NROW, 128), np.float32)
    TRIZBA = np.zeros((NROW, 128), np.float32)
    for g in range(G):
        for i in range(SH):
            TRIZA[25 * g:25 * g + i, 25 * g + i] = 1.0
            TRIZB[25 * g:25 * g + i, 25 * g + i] = 1.0
            TRIZBA[25 * g:25 * g + 25, 25 * g + i] = 1.0

    # contraction: out row g = Y-partial of group g, row 5+g = S50
    CBA = np.zeros((3, NROW, 32), np.float32)
    CBB = np.zeros((3, NROW, 32), np.float32)
    XPB = np.zeros((NROW, 32), np.float32)
    z1 = _zeta_np(np.ones(1), ts[0], W1, b1, W2, b2, W3, b3)[0]
    for g in range(G):
        for i in range(SH):
            if i == 0:
                # x_0 == 1 exactly: fold gam_0 * z(1,t_0) into the dw tile
                CBA[0, 25 * g + 0, g] = gam[0] * z1
            else:
                for k in range(3):
                    CBA[k, 25 * g + i, g] = gam[i] * beta[i, k]
            for k in range(3):
                CBB[k, 25 * g + i, g] = gam[25 + i] * beta[25 + i, k]
        CBB[1, 25 * g + 24, 5 + g] = S0 * SIGMA   # S50 = S0*(a*x49 + s*w49)
        XPB[25 * g + 24, 5 + g] = S0 * A

    c = {"TRIZA": TRIZA, "TRIZB": TRIZB, "TRIZBA": TRIZBA,
         "CBA0": CBA[0], "CBA1": CBA[1], "CBA2": CBA[2],
         "CBB0": CBB[0], "CBB1": CBB[1], "CBB2": CBB[2], "XPB": XPB}
    return {k: v.astype(ml_dtypes.bfloat16) for k, v in c.items()}


CDEFS = [("TRIZA", [NROW, 128]), ("TRIZB", [NROW, 128]),
         ("TRIZBA", [NROW, 128]),
         ("CBA0", [NROW, 32]), ("CBA1", [NROW, 32]), ("CBA2", [NROW, 32]),
         ("CBB0", [NROW, 32]), ("CBB1", [NROW, 32]), ("CBB2", [NROW, 32]),
         ("XPB", [NROW, 32])]


def _build_kernel(num_devices, nreps=1):
    nc = bacc.Bacc("TRN2", debug=False, num_devices=num_devices,
                   target_bir_lowering=False)

    # Ln+Exp live together in "natural_log_exp_and_others"; pin that one
    # table for the whole program to avoid dynamic table reloads.
    from concourse.hw_specs import get_activation_tables
    import concourse.bacc as _bacc_mod

    def _pinned_act_table_loads():
        tables = []
        for name, funcs in get_activation_tables(nc.m.arch).items():
            tables.append((name, funcs if name == "natural_log_exp_and_others"
                           else set()))
        _bacc_mod._bass_rust.insert_act_table_loads(nc, tables)

    nc.insert_act_table_loads = _pinned_act_table_loads
    tc = tile.TileContext(nc)

    dwTA = nc.dram_tensor("dwTA", [128, NCOL], BF16, kind="ExternalInput")
    dwTB = nc.dram_tensor("dwTB", [128, NCOL], BF16, kind="ExternalInput")
    cins = {n: nc.dram_tensor(n, s, BF16, kind="ExternalInput")
            for n, s in CDEFS}
    lbin = nc.dram_tensor("LB", [128, 1], F32, kind="ExternalInput")
    Zout = nc.dram_tensor("Zout", [NQ * 128, 512], F32, kind="ExternalOutput")

    NT = 2 * NQ                     # quad-slabs per pass (A/B interleaved)

    with tc:
        with tc.tile_pool(name="consts", bufs=1) as cpool, \
             tc.tile_pool(name="inp", bufs=4) as ipool, \
             tc.tile_pool(name="lgp", bufs=4) as lpool, \
             tc.tile_pool(name="xwp", bufs=LQ + 2) as xpool, \
             tc.tile_pool(name="p1p", bufs=2) as ppool, \
             tc.tile_pool(name="stg", bufs=2) as spool, \
             tc.tile_pool(name="ps_pref", bufs=3, space="PSUM") as p_pref, \
             tc.tile_pool(name="ps_y", bufs=2, space="PSUM") as p_y:

            C = {}
            for n, s in CDEFS:
                C[n] = cpool.tile(s, BF16, name=f"c_{n}", tag=f"c_{n}")
                nc.sync.dma_start(C[n][:], cins[n].ap())
            LB = cpool.tile([128, 1], F32, name="c_LB", tag="c_LB")
            nc.sync.dma_start(LB[:], lbin.ap())

            dwt = {}
            lg = {}
            xq = {}
            wq = {}
            p2q = {}
            ypt = {}

            def dma_in(j):
                dwt[2 * j] = ipool.tile([128, 2048], BF16, name="dwa",
                                        tag="dwa")
                dwt[2 * j + 1] = ipool.tile([128, 2048], BF16, name="dwb",
                                            tag="dwb")
                cols = slice(2048 * j, 2048 * (j + 1))
                nc.sync.dma_start(dwt[2 * j][:], dwTA.ap()[:, cols])
                nc.sync.dma_start(dwt[2 * j + 1][:], dwTB.ap()[:, cols])

            def ln_op(t):
                lg[t] = lpool.tile([128, 2048], BF16, name="lg", tag="lg")
                nc.scalar.activation(lg[t][:], dwt[t][:], AF.Ln,
                                     bias=LB[:], scale=SIGMA)

            def front(t):
                ph = t % 2
                if t + 1 < NT:
                    ln_op(t + 1)
                TRZ = C["TRIZB"] if ph else C["TRIZA"]
                x = xpool.tile([128, 2048], BF16, name="x", tag="x")
                for h in range(2):
                    pref = p_pref.tile([128, 1024], F32, name="pref",
                                       tag="pref")
                    for k2 in range(2):
                        cols = slice(1024 * h + 512 * k2,
                                     1024 * h + 512 * (k2 + 1))
                        oc = slice(512 * k2, 512 * (k2 + 1))
                        nc.tensor.matmul(pref[:, oc], TRZ[:],
                                         lg[t][0:NROW, cols],
                                         start=True, stop=(ph == 0))
                        if ph == 1:
                            nc.tensor.matmul(pref[:, oc], C["TRIZBA"][:],
                                             lg[t - 1][0:NROW, cols],
                                             start=False, stop=True)
                    nc.scalar.activation(x[:, 1024 * h:1024 * (h + 1)],
                                         pref[:], AF.Exp)
                w = xpool.tile([128, 2048], BF16, name="w", tag="w")
                nc.vector.tensor_tensor(w[:], x[:], dwt[t][:], op=ALU.mult)
                p1 = ppool.tile([128, 2048], BF16, name="p1", tag="p1")
                nc.vector.tensor_tensor(p1[:], w[:], x[:], op=ALU.mult)
                p2 = xpool.tile([128, 2048], BF16, name="p2", tag="p2")
                nc.vector.tensor_tensor(p2[:], p1[:], x[:], op=ALU.mult)
                xq[t], wq[t], p2q[t] = x, w, p2

            def contract(t):
                j, ph = t // 2, t % 2
                if ph == 0:
                    ypt[j] = p_y.tile([128, 512], F32, name="yp", tag="yp")
                yp = ypt[j]
                cb = ("CBB0", "CBB1", "CBB2") if ph else \
                     ("CBA0", "CBA1", "CBA2")
                for jj in range(4):
                    cols = slice(512 * jj, 512 * (jj + 1))
                    ys = yp[32 * jj:32 * (jj + 1), :]
                    tp = (0, 32 * jj)
                    nc.tensor.matmul(ys, C[cb[0]][:], dwt[t][0:NROW, cols],
                                     start=(ph == 0), stop=False,
                                     tile_position=tp)
                    nc.tensor.matmul(ys, C[cb[1]][:], wq[t][0:NROW, cols],
                                     start=False, stop=False,
                                     tile_position=tp)
                    nc.tensor.matmul(ys, C[cb[2]][:], p2q[t][0:NROW, cols],
                                     start=False, stop=(ph == 1 and False),
                                     tile_position=tp)
                    if ph == 1:
                        nc.tensor.matmul(ys, C["XPB"][:],
                                         xq[t][0:NROW, cols],
                                         start=False, stop=True,
                                         tile_position=tp)
                del xq[t], wq[t], p2q[t]
                if ph == 1:
                    stg = spool.tile([128, 512], F32, name="st", tag="st")
                    nc.vector.tensor_copy(stg[:], yp[:])
                    nc.sync.dma_start(Zout.ap()[128 * j:128 * (j + 1), :],
                                      stg[:])

            for rep in range(nreps):
                dma_in(0)
                dma_in(1)
                ln_op(0)
                for t in range(NT + LQ):
                    if t < NT:
                        j, ph = t // 2, t % 2
                        if ph == 0 and j + 2 < NQ:
                            dma_in(j + 2)
                        front(t)
                    if t >= LQ:
                        contract(t - LQ)

    nc.compile()
    return nc


_CACHE = {}
_LAST_IN_MAPS = None


def kernel(dw, t_grid, W1, b1, W2, b2, W3, b3, Y0):
    import ml_dtypes
    dw = np.asarray(dw, np.float32)
    t_grid = np.asarray(t_grid, np.float32)
    B = dw.shape[0]
    assert B == B_FULL and dw.shape[1] == M
    a50y0 = np.float32(A ** M * np.float32(Y0))

    if "nc" not in _CACHE:
        _CACHE["nc"] = _build_kernel(NCORES)
    nc = _CACHE["nc"]

    consts = _build_consts(np.asarray(W1, np.float32),
                           np.asarray(b1, np.float32),
                           np.asarray(W2, np.float32),
                           np.asarray(b2, np.float32),
                           np.asarray(W3, np.float32),
                           np.asarray(b3, np.float32), t_grid[0])

    dwb = dw.astype(ml_dtypes.bfloat16)
    in_maps = []
    for ci in range(NCORES):
        blk = dwb[ci * B_CORE:(ci + 1) * B_CORE]
        pad = np.zeros((G * NCOL, M), ml_dtypes.bfloat16)
        pad[:B_CORE] = blk
        v = pad.reshape(G, NCOL, M)
        dwTA = np.zeros((128, NCOL), ml_dtypes.bfloat16)
        dwTB = np.zeros((128, NCOL), ml_dtypes.bfloat16)
        dwTA[0:NROW] = v[:, :, 0:SH].transpose(0, 2, 1).reshape(NROW, NCOL)
        dwTB[0:NROW] = v[:, :, SH:M].transpose(0, 2, 1).reshape(NROW, NCOL)
        mci = dict(consts)
        mci["dwTA"] = dwTA
        mci["dwTB"] = dwTB
        mci["LB"] = np.full((128, 1), A, np.float32)
        in_maps.append(mci)

    global _LAST_IN_MAPS
    _LAST_IN_MAPS = in_maps
    res = bass_utils.run_bass_kernel_spmd(nc, in_maps,
                                          core_ids=list(range(NCORES)))

    Y = np.empty((B_FULL,), np.float32)
    S = np.empty((B_FULL,), np.float32)
    for ci in range(NCORES):
        # Zout row 128j + 32jj + q: q in 0..4 -> Y of group q, 5..9 -> S50;
        # column c of that row -> path (q%5)*NCOL + 2048j + 512jj + c
        Z = res.results[ci]["Zout"].reshape(NQ, 4, 32, 512)
        Ymap = Z[:, :, 0:G, :].transpose(2, 0, 1, 3).reshape(G * NCOL)
        Smap = Z[:, :, G:2 * G, :].transpose(2, 0, 1, 3).reshape(G * NCOL)
        Y[ci * B_CORE:(ci + 1) * B_CORE] = Ymap[:B_CORE]
        S[ci * B_CORE:(ci + 1) * B_CORE] = Smap[:B_CORE]
    Y += a50y0
    return Y[:, None], S[:, None]
